# revision 58
# baseline (speedup 1.0000x reference)
"""Trainium2 Bass kernel for nn_CrossAttention (dense_transformer).

Reference computation (per batch b, per stream s in {1,2}):
    q_s   = heads(x_s)                      # [H, N, D] slices of x_s
    kv_s  = x_s @ Wkv_s -> k_s, v_s         # [N, C] each
    gate_s= sigmoid(relu(x_s @ w1 + b1) @ w2 + b2)
    ctx_s = softmax_d( scale * k_s^T @ (v_s * gate_s) )   # [H, D, D]
    o_1   = q_1 @ ctx_2 ; o_2 = q_2 @ ctx_1  (cross)

Sharding: 8 cores = (stream s, batch b) pairs.  Core (s, b) projects
x_s[b] (kv + gate + ctx_s[b]) and then computes the OTHER stream's
output o_{1-s}[b] = q_{1-s}[b] @ softmax(ctx_s[b]).  No cross-core
communication; host concatenates outputs.

v2: host pre-transposes/pre-casts x (fp16), so the device does no
transposes and no DRAM spills.  All GEMMs fp16 (1 cycle/row); the gate
MLP can optionally run fp8e4 DoubleRow (0.5 cycles/row).  ctx is
accumulated in PSUM across all 32 n-chunks (two 8-head groups stacked
on partition halves -> one PSUM bank).
"""

import numpy as np
from contextlib import ExitStack

N = 4096
C = 1024
H = 16
D = 64
SCALE = D ** (-0.5)
NCH = N // 128       # 32 n-chunks of 128 rows

GATE_MODE = "fp8"    # 'fp16' | 'fp8'
S_X = 16.0           # fp8 activation scale for x
S_W = 256.0          # fp8 weight scale
S_H = 32.0           # fp8 scale for hidden h

_CACHE = {}


def _build_program(gate_mode, with_bias):
    import concourse.bass as bass
    import concourse.bacc as bacc
    import concourse.tile as tile
    import concourse.mybir as mybir

    F32 = mybir.dt.float32
    FP16 = mybir.dt.float16
    FP8 = mybir.dt.float8e4
    AF = mybir.ActivationFunctionType
    DR = mybir.MatmulPerfMode.DoubleRow
    fp8 = gate_mode == "fp8"
    HDT = FP8 if fp8 else FP16

    nc = bacc.Bacc("TRN2", target_bir_lowering=False, debug=False, num_devices=8)

    # weights arrive host-rearranged to the SBUF layout [p, k, m] so the
    # DMA is a straight contiguous copy (8-16KB lines per partition)
    xt = nc.dram_tensor("xt", [C, N], FP16, kind="ExternalInput").ap()
    xqt = nc.dram_tensor("xqt", [C, N], FP16, kind="ExternalInput").ap()
    wkv = nc.dram_tensor("wkv", [128, 8 * 2 * C], FP16, kind="ExternalInput").ap()
    w1 = nc.dram_tensor("w1", [128, 8 * C], HDT, kind="ExternalInput").ap()
    w2 = nc.dram_tensor("w2", [128, 8 * C], HDT, kind="ExternalInput").ap()
    b1s = nc.dram_tensor("b1s", [128, 8], F32, kind="ExternalInput").ap()
    ident = nc.dram_tensor("ident", [128, 64], F32, kind="ExternalInput").ap()
    if fp8:
        xt8 = nc.dram_tensor("xt8", [C, N], FP8, kind="ExternalInput").ap()
    if with_bias:
        b2r = nc.dram_tensor("b2r", [1, C], FP16, kind="ExternalInput").ap()
    # output in TILED layout: piece (j, pi) = [128 c-dims of pair j,
    # 1024 n-cols] stored fully contiguous (256KB DRAM bursts instead of
    # 2KB rows strided 8KB -> better write bandwidth); host reassembles.
    ot = nc.dram_tensor("ot", [N, C], FP16, kind="ExternalOutput").ap()
    # tiny sink so the act-table prewarm exps aren't dead-code eliminated
    wsink = nc.dram_tensor("wsink", [1, 16], F32, kind="ExternalOutput").ap()

    # activation post-scales to undo the fp8 pre-scales
    g1_scale = (S_H / (S_X * S_W)) if fp8 else 1.0
    g2_scale = (1.0 / (S_H * S_W)) if fp8 else 1.0
    ones_val = (S_H * S_W) if fp8 else 1.0

    with tile.TileContext(nc) as tc, ExitStack() as ctx:
        # ---------- persistent constants ----------
        # DMA order matters: only w1/b1 (+ first x block) gate the first
        # matmul, so emit those first and defer w2/wkv into block 0's
        # compute window.
        # big DMAs are split into k-range parts so they spread across DMA
        # queues (a single dma_start lands on one queue)
        def dma_split(dst, src, parts):
            kk = 8 // parts
            for i in range(parts):
                nc.sync.dma_start(
                    dst[:, i * kk:(i + 1) * kk, :], src[:, i * kk:(i + 1) * kk, :]
                )

        cpool = ctx.enter_context(tc.tile_pool(name="consts", bufs=1))
        w1_sb = cpool.tile([128, 8, C], HDT, name="w1_sb")
        b1_sb = cpool.tile([128, 8], F32, name="b1_sb")
        w2_sb = cpool.tile([128, 8, C], HDT, name="w2_sb")
        wkv_sb = cpool.tile([128, 8, 2 * C], FP16, name="wkv_sb")
        ident_sb = cpool.tile([128, 64], F32, name="ident_sb")

        def emit_deferred_consts():
            dma_split(w2_sb, w2.rearrange("p (k m) -> p k m", k=8), 4)
            dma_split(wkv_sb, wkv.rearrange("p (k m) -> p k m", k=8), 8)
            nc.sync.dma_start(ident_sb, ident)
        if with_bias:
            ones_sb = cpool.tile([1, 128], F32, name="ones_sb")
            nc.vector.memset(ones_sb, ones_val)
            ones_r = cpool.tile([1, 128], FP16, name="ones_r")
            nc.vector.tensor_copy(ones_r, ones_sb)
            b2_r = cpool.tile([1, C], FP16, name="b2_r")
            nc.sync.dma_start(b2_r, b2r)

        # pre-warm the scalar Exp activation table while the engine is idle
        # so the softmax doesn't pay the ~1.3us ACT_TABLE_LOAD at its start
        warm = cpool.tile([1, 16], F32, name="warm")
        nc.vector.memset(warm, 0.0)
        warm2 = cpool.tile([1, 16], F32, name="warm2")
        nc.scalar.activation(warm2, warm, AF.Exp)

        # spair off-diagonal blocks are zero forever: zero them once here,
        # off the critical path, so softmax only writes the diagonals
        spool = ctx.enter_context(tc.tile_pool(name="spairs", bufs=1))
        spairs = [spool.tile([128, 128], FP16, name=f"spair{j}") for j in range(8)]
        for j in range(8):
            nc.vector.memset(spairs[j], 0.0)

        # ctx accumulator in PSUM: heads 0-7 on partitions 0-63, heads
        # 8-15 on 64-127; head h at cols (h%8)*64, layout [e, d].
        ctxps_pool = ctx.enter_context(
            tc.tile_pool(name="ctxps", bufs=1, space="PSUM")
        )
        ctx_ps = ctxps_pool.tile([128, 512], F32, name="ctx_ps")

        # phase-B xq tiles live alongside phase A so DMA prefetch overlaps
        bxq_pool = ctx.enter_context(tc.tile_pool(name="bxq", bufs=4))

        def emit_bxq_dma(blk):
            bx = bxq_pool.tile([128, 8, 1024], FP16, name="bx", tag="bx")
            src = xqt.rearrange("(j p) n -> p j n", p=128)[
                :, :, blk * 1024:(blk + 1) * 1024
            ]
            for i in range(4):
                nc.sync.dma_start(bx[:, 2 * i:2 * i + 2, :], src[:, 2 * i:2 * i + 2, :])
            return bx

        # =========================================================
        # Phase A: gates + kv projection + ctx accumulation, fused
        # =========================================================
        with ExitStack() as pa:
            xt_pool = pa.enter_context(tc.tile_pool(name="xt", bufs=2))
            if fp8:
                xt8_pool = pa.enter_context(tc.tile_pool(name="xt8", bufs=2))
            ht_pool = pa.enter_context(tc.tile_pool(name="ht", bufs=2))
            g_pool = pa.enter_context(tc.tile_pool(name="g", bufs=3))
            kf_pool = pa.enter_context(tc.tile_pool(name="kf", bufs=4))
            vg_pool = pa.enter_context(tc.tile_pool(name="vg", bufs=4))
            gps_pool = pa.enter_context(
                tc.tile_pool(name="gps", bufs=3, space="PSUM")
            )
            kvps_pool = pa.enter_context(
                tc.tile_pool(name="kvps", bufs=2, space="PSUM")
            )

            bx_tiles = {}
            pending = []  # (kf, vg, global_chunk) awaiting ctx matmuls

            def emit_ctx(kf_t, vg_t, gc):
                # start=True marks the whole 2KB PSUM bank (per partition)
                # as pending-zero, so issue it exactly once per partition
                # half; the other heads' first writes then init via the
                # pending-zero overwrite instead of accumulating garbage.
                # Head order puts the softmax's first column stage (ctx cols
                # 0:256 = heads 0-3 and 8-11) first so its reduce can start
                # before the remaining heads' final matmuls land.  h=0 and
                # h=8 stay first within their partition halves, preserving
                # the start-flag invariant.
                for h in (0, 1, 2, 3, 8, 9, 10, 11, 4, 5, 6, 7, 12, 13, 14, 15):
                    nc.tensor.matmul(
                        ctx_ps[
                            (h // 8) * 64:(h // 8) * 64 + 64,
                            (h % 8) * 64:(h % 8) * 64 + 64,
                        ],
                        vg_t[:, h * D:(h + 1) * D],
                        kf_t[:, h * D:(h + 1) * D],
                        start=(gc == 0 and h % 8 == 0),
                        stop=(gc == NCH - 1),
                        skip_group_check=True,
                    )

            def emit_inputs(blk):
                # input DMAs split by n-half so the first matmul of the
                # block is gated on half the bytes; at blk 0 interleave the
                # w1/b1 emissions so all gating loads land on the earliest-
                # starting DMA queues.  Quarter-sized parts: the DMA
                # pipeline has a fixed ~8us startup latency and bigger
                # pieces ramp its throughput faster than many small ones;
                # all dynamic queues share one DMA engine, so everything
                # stays on sync's queue in consumption order.
                xt8_t = None
                if fp8:
                    xt8_t = xt8_pool.tile([128, 8, C], FP8, name="xt8_in", tag="xt8")
                    src8 = xt8.rearrange("(k p) n -> p k n", p=128)
                    w1m = w1.rearrange("p (k m) -> p k m", k=8)
                    # piece order matches gate1's all-m-over-half0 sweep:
                    # that sweep consumes xt8-h0 AND all four w1 quarters
                    # in its first 6.8us, while xt8-h1 isn't needed until
                    # the second sweep -- so h1 pieces go last at blk 0
                    def xt8_piece(kq, hf):
                        nc.sync.dma_start(
                            xt8_t[:, kq * 4:(kq + 1) * 4,
                                  hf * 512:(hf + 1) * 512],
                            src8[:, kq * 4:(kq + 1) * 4,
                                 blk * 1024 + hf * 512:
                                 blk * 1024 + (hf + 1) * 512],
                        )

                    def w1_piece(i):
                        nc.sync.dma_start(
                            w1_sb[:, :, i * 256:(i + 1) * 256],
                            w1m[:, :, i * 256:(i + 1) * 256],
                        )

                    if blk == 0:
                        w1_piece(0)
                        xt8_piece(0, 0)
                        xt8_piece(1, 0)
                        # quarters 1-3 as ONE 768KB piece, issued before b1:
                        # bigger pieces ramp the cold DMA pipeline faster,
                        # and m2-m7 of the first sweep all gate on it
                        nc.sync.dma_start(w1_sb[:, :, 256:1024],
                                          w1m[:, :, 256:1024])
                        nc.sync.dma_start(b1_sb, b1s)
                        xt8_piece(0, 1)
                        xt8_piece(1, 1)
                    else:
                        for hf in range(2):
                            for kq in range(2):
                                xt8_piece(kq, hf)
                elif blk == 0:
                    dma_split(w1_sb, w1.rearrange("p (k m) -> p k m", k=8), 8)
                    nc.sync.dma_start(b1_sb, b1s)
                xt_t = xt_pool.tile([128, 8, C], FP16, name="xt_in", tag="xt")
                srcx = xt.rearrange("(k p) n -> p k n", p=128)
                for hf in range(2):
                    for kq in range(2):
                        nc.sync.dma_start(
                            xt_t[:, kq * 4:(kq + 1) * 4,
                                 hf * 512:(hf + 1) * 512],
                            srcx[:, kq * 4:(kq + 1) * 4,
                                 blk * 1024 + hf * 512:
                                 blk * 1024 + (hf + 1) * 512],
                        )
                return xt8_t, xt_t

            in_tiles = {0: emit_inputs(0)}
            for blk in range(4):
                xt8_in, xt_in = in_tiles.pop(blk)

                # ---- gate1: hT[m-tile, n] = relu(x@w1+b1).T ----
                # half OUTER ACROSS m: the whole m-sweep for n-half 0 runs
                # on the first 512KB of xt8 (6.8us of compute) while the
                # h1 pieces stream in behind it -> no mid-gate1 DMA stall
                ht = ht_pool.tile([128, 8, C], HDT, name="ht", tag="ht")
                for half in range(2):
                    for m in range(8):
                        ps1 = gps_pool.tile([128, 512], F32, name="g1ps",
                                            tag="gps")
                        if fp8:
                            for kp in range(4):
                                nc.tensor.matmul(
                                    ps1,
                                    w1_sb[:, 2 * kp:2 * kp + 2,
                                          m * 128:(m + 1) * 128],
                                    xt8_in[:, 2 * kp:2 * kp + 2,
                                           half * 512:(half + 1) * 512],
                                    start=(kp == 0),
                                    stop=(kp == 3),
                                    perf_mode=DR,
                                )
                        else:
                            for k in range(8):
                                nc.tensor.matmul(
                                    ps1,
                                    w1_sb[:, k, m * 128:(m + 1) * 128],
                                    xt_in[:, k, half * 512:(half + 1) * 512],
                                    start=(k == 0),
                                    stop=(k == 7),
                                )
                        nc.scalar.activation(
                            ht[:, m, half * 512:(half + 1) * 512],
                            ps1,
                            AF.Relu,
                            bias=b1_sb[:, m:m + 1],
                            scale=g1_scale,
                        )

                if blk == 0:
                    # w2/wkv arrive during block 0's gate1; xq prefetches after
                    emit_deferred_consts()
                # prefetch phase-B xq tiles while DMA is quiet
                if blk >= 1:
                    bx_tiles[blk - 1] = emit_bxq_dma(blk - 1)
                    if blk == 3:
                        bx_tiles[3] = emit_bxq_dma(3)

                # ---- per chunk: gate2 -> kv -> (delayed) ctx ----
                for c in range(8):
                    gc = blk * 8 + c
                    if c == 3 and blk < 3:
                        # prefetch the next block's inputs from mid-block so
                        # its gate1 never waits on the just-issued DMAs
                        in_tiles[blk + 1] = emit_inputs(blk + 1)
                    gt = g_pool.tile([128, C], FP16, name="gt", tag="gt")
                    for t in range(2):
                        ps2 = gps_pool.tile([128, 512], F32, name="g2ps", tag="gps")
                        if fp8:
                            for kp in range(4):
                                nc.tensor.matmul(
                                    ps2,
                                    ht[:, 2 * kp:2 * kp + 2, c * 128:(c + 1) * 128],
                                    w2_sb[:, 2 * kp:2 * kp + 2,
                                          t * 512:(t + 1) * 512],
                                    start=(kp == 0),
                                    stop=(kp == 3 and not with_bias),
                                    perf_mode=DR,
                                )
                        else:
                            for k in range(8):
                                nc.tensor.matmul(
                                    ps2,
                                    ht[:, k, c * 128:(c + 1) * 128],
                                    w2_sb[:, k, t * 512:(t + 1) * 512],
                                    start=(k == 0),
                                    stop=(k == 7 and not with_bias),
                                )
                        if with_bias:
                            nc.tensor.matmul(
                                ps2,
                                ones_r,
                                b2_r[:, t * 512:(t + 1) * 512],
                                start=False,
                                stop=True,
                            )
                        nc.scalar.activation(
                            gt[:, t * 512:(t + 1) * 512], ps2, AF.Sigmoid,
                            scale=g2_scale,
                        )
                    if gc == NCH - 1:
                        # last sigmoid just ran: the act-table switch to the
                        # exp set (~1.3us) would otherwise land on the softmax
                        # critical path.  Force it now, hidden under the last
                        # chunk's kv matmuls (relu/copy/exp share a table set;
                        # only sigmoid conflicts).  Reading gt (the last
                        # sigmoid's output) pins this after that sigmoid in
                        # the schedule; the DMA sink defeats dead-code elim.
                        nc.scalar.activation(warm2, gt[0:1, 1008:1024], AF.Exp)
                        nc.sync.dma_start(wsink, warm2)

                    # kv projection for this chunk; k and v psum halves
                    ps_k = kvps_pool.tile([128, C], F32, name="ps_k", tag="kvps")
                    ps_v = kvps_pool.tile([128, C], F32, name="ps_v", tag="kvps")
                    for k in range(8):
                        lhs = xt_in[:, k, c * 128:(c + 1) * 128]
                        for t in range(2):
                            nc.tensor.matmul(
                                ps_k[:, t * 512:(t + 1) * 512],
                                lhs,
                                wkv_sb[:, k, t * 512:(t + 1) * 512],
                                start=(k == 0),
                                stop=(k == 7),
                            )
                        for t in range(2):
                            nc.tensor.matmul(
                                ps_v[:, t * 512:(t + 1) * 512],
                                lhs,
                                wkv_sb[:, k, C + t * 512:C + (t + 1) * 512],
                                start=(k == 0),
                                stop=(k == 7),
                            )
                    kf = kf_pool.tile([128, C], FP16, name="kf", tag="kf")
                    nc.scalar.copy(kf, ps_k)
                    vg = vg_pool.tile([128, C], FP16, name="vg", tag="vg")
                    if gc == NCH - 1:
                        # split the final vg mul so the last ctx pair (heads
                        # 0-7 need only cols 0:512) starts ~0.6us earlier
                        nc.vector.tensor_mul(vg[:, 0:512], ps_v[:, 0:512],
                                             gt[:, 0:512])
                        nc.vector.tensor_mul(vg[:, 512:1024], ps_v[:, 512:1024],
                                             gt[:, 512:1024])
                    else:
                        nc.vector.tensor_mul(vg, ps_v, gt)

                    # ctx lags a chunk and is emitted in pairs: the kf/vg
                    # conversions overlap the kv matmuls, and batching two
                    # chunks of tiny ctx matmuls halves the PE tile-mode
                    # transitions (~200ns each); lag kept minimal so the
                    # post-loop drain is at most two chunks
                    pending.append((kf, vg, gc))
                    if len(pending) >= 3:
                        emit_ctx(*pending.pop(0))
                        emit_ctx(*pending.pop(0))

            while pending:
                emit_ctx(*pending.pop(0))

        # =========================================================
        # Softmax over d (free dim of ctxT) + block-diag S pairs
        # =========================================================
        with ExitStack() as sm:
            smp = sm.enter_context(tc.tile_pool(name="smpool", bufs=1))
            # 4 bufs (not 7): leaves 3 PSUM banks free so phase B's first
            # matmuls aren't blocked on the spair copies draining these
            smps = sm.enter_context(tc.tile_pool(name="smps", bufs=4, space="PSUM"))
            maxs = smp.tile([128, 8], F32, name="maxs")
            cmx = smp.tile([128, 512], F32, name="cmx")
            et = smp.tile([128, 512], F32, name="et")
            sums = smp.tile([128, 8], F32, name="sums")
            recs = smp.tile([128, 8], F32, name="recs")
            st = smp.tile([128, 512], F32, name="st")
            # softmax runs in two column halves so the first transposes
            # (pairs 0,1,4,5 live in cols 0:256) start ~2us earlier
            for c0, c1, pairs in ((0, 256, (0, 1, 4, 5)),
                                  (256, 512, (2, 3, 6, 7))):
                cs = slice(c0, c1)
                hs = slice(c0 // 64, c1 // 64)
                b = (c1 - c0) // 64
                nc.vector.tensor_reduce(
                    maxs[:, hs],
                    ctx_ps[:, cs].rearrange("p (b d) -> p b d", b=b),
                    axis=mybir.AxisListType.X,
                    op=mybir.AluOpType.max,
                )
                nc.vector.tensor_sub(
                    cmx[:, cs].rearrange("p (h d) -> p h d", h=b),
                    ctx_ps[:, cs].rearrange("p (h d) -> p h d", h=b),
                    maxs[:, hs].unsqueeze(-1).broadcast_to([128, b, 64]),
                )
                nc.scalar.activation(et[:, cs], cmx[:, cs], AF.Exp,
                                     scale=float(SCALE))
                nc.vector.tensor_reduce(
                    sums[:, hs],
                    et[:, cs].rearrange("p (b d) -> p b d", b=b),
                    axis=mybir.AxisListType.X,
                    op=mybir.AluOpType.add,
                )
                nc.vector.reciprocal(recs[:, hs], sums[:, hs])
                nc.vector.tensor_mul(
                    st[:, cs].rearrange("p (h d) -> p h d", h=b),
                    et[:, cs].rearrange("p (h d) -> p h d", h=b),
                    recs[:, hs].unsqueeze(-1).broadcast_to([128, b, 64]),
                )
            # st rows e (64 per half), cols d per head.  Transposing the
            # side-by-side pair [ctxT_2j | ctxT_2j+1] ([64, 128]) gives
            # [S_2j stacked above S_2j+1] ([128, 64]); scatter block-diag.
            # Both copies of a pair stay on ONE engine (cross-engine writes
            # into the same tile raced); pair order follows softmax halves.
            for i, j in enumerate((0, 1, 4, 5, 2, 3, 6, 7)):
                half = j // 4  # heads 0-7 lower partitions, 8-15 upper
                base = half * 64
                colj = (2 * j) % 8
                tp = smps.tile([128, 64], F32, name="smtp", tag="smtp")
                nc.tensor.transpose(
                    tp,
                    st[base:base + 64, colj * 64:(colj + 2) * 64],
                    ident_sb[base:base + 64, :],
                )
                if i % 2 == 0:
                    nc.vector.tensor_copy(spairs[j][0:64, 0:64], tp[0:64, :])
                    nc.vector.tensor_copy(spairs[j][64:128, 64:128], tp[64:128, :])
                else:
                    nc.scalar.copy(spairs[j][0:64, 0:64], tp[0:64, :])
                    nc.scalar.copy(spairs[j][64:128, 64:128], tp[64:128, :])

        # =========================================================
        # Phase B: ot[j*128:(j+1)*128, n] = blockdiag(S_j)^T @ q_pair
        # S stationary per j: 8 matmuls of free=512 each (vs 256 tiny
        # ones), paced by PSUM->SBUF copies spread over three engines.
        # =========================================================
        with ExitStack() as pb:
            oj_pool = pb.enter_context(tc.tile_pool(name="bo", bufs=3))
            bops_pool = pb.enter_context(
                tc.tile_pool(name="bops", bufs=7, space="PSUM")
            )
            ei = 0
            # GPSIMD cannot read PSUM on TRN2: strictly alternate the two
            # engines that can, so each (vector, scalar) pair of copies
            # completes in one copy-time and feeds the output DMA steadily
            for j in (0, 1, 4, 5, 2, 3, 6, 7):
                oj = oj_pool.tile([128, N], FP16, name="oj", tag="oj")
                for nb in range(8):
                    blk, half = nb // 2, nb % 2
                    ops = bops_pool.tile([128, 512], F32, name="ops", tag="ops")
                    nc.tensor.matmul(
                        ops,
                        spairs[j],
                        bx_tiles[blk][:, j, half * 512:(half + 1) * 512],
                        start=True,
                        stop=True,
                        skip_group_check=True,
                    )
                    # 34:30 vector:scalar — scalar's copies slow from 687 to
                    # ~880ns over phase B, so shift two late-j copies to the
                    # vector engine to keep the two queues draining together.
                    # j0 starts on SCALAR: vector is still draining softmax
                    # stage-2 ops + spair copies, scalar is free after its
                    # exps, so the first output piece forms ~1us earlier.
                    if j == 0:
                        use_vec = ei % 2 == 1
                    else:
                        use_vec = ei % 2 == 0 or (j in (6, 7) and nb == 7)
                    if use_vec:
                        nc.vector.tensor_copy(oj[:, nb * 512:(nb + 1) * 512], ops)
                    else:
                        nc.scalar.copy(oj[:, nb * 512:(nb + 1) * 512], ops)
                    ei += 1
                    if nb % 4 == 3:
                        # 512KB output pieces: sync's sequencer issues one
                        # DMA per ~1us, so 16 pieces keep issue capacity
                        # (~512GB/s) above the ~300GB/s write bandwidth;
                        # each piece lands contiguous in DRAM (tiled layout)
                        pi = j * 2 + nb // 4
                        nc.sync.dma_start(
                            ot[pi * 256:(pi + 1) * 256, :].rearrange(
                                "(p a) n -> p (a n)", a=2),
                            oj[:, (nb - 3) * 512:(nb + 1) * 512],
                        )

    nc.compile()
    return nc


def _get_program(gate_mode=None, with_bias=False):
    if gate_mode is None:
        gate_mode = GATE_MODE
    key = (gate_mode, bool(with_bias))
    if key not in _CACHE:
        _CACHE[key] = _build_program(gate_mode, with_bias)
    return _CACHE[key]


def make_in_maps(x1, x2, Wkv1, Wkv2, g1_w1, g1_b1, g1_w2, g1_b2,
                 g2_w1, g2_b1, g2_w2, g2_b2, gate_mode=None):
    """Core (s, b): cores 0-3 = (s=0, b), cores 4-7 = (s=1, b)."""
    import ml_dtypes
    if gate_mode is None:
        gate_mode = GATE_MODE
    fp8 = gate_mode == "fp8"
    F8 = ml_dtypes.float8_e4m3
    ident = np.vstack([np.eye(64, dtype=np.float32)] * 2)

    def dev_w(w):
        # [k*128+p, m] -> [p, k*M+m] (SBUF layout, contiguous DMA lines)
        M = w.shape[1]
        return np.ascontiguousarray(
            w.reshape(8, 128, M).transpose(1, 0, 2).reshape(128, 8 * M)
        )

    def prep_stream(x, wkv, w1, b1, w2, b2):
        m = {
            "xt": x.T.astype(np.float16, order="C"),
            "wkv": dev_w(wkv.astype(np.float16)),
            "ident": ident,
        }
        if fp8:
            m["xt8"] = (x.T * S_X).astype(F8, order="C")
            m["w1"] = dev_w((w1 * S_W).astype(F8))
            m["w2"] = dev_w((w2 * S_W).astype(F8))
            m["b1s"] = np.ascontiguousarray((S_H * b1).reshape(8, 128).T)
        else:
            m["w1"] = dev_w(w1.astype(np.float16))
            m["w2"] = dev_w(w2.astype(np.float16))
            m["b1s"] = np.ascontiguousarray(b1.reshape(8, 128).T)
        m["b2r"] = b2.reshape(1, C).astype(np.float16)
        return m

    in_maps = []
    for core in range(8):
        s, b = core // 4, core % 4
        if s == 0:
            m = prep_stream(x1[b], Wkv1, g1_w1, g1_b1, g1_w2, g1_b2)
            m["xqt"] = x2[b].T.astype(np.float16, order="C")
        else:
            m = prep_stream(x2[b], Wkv2, g2_w1, g2_b1, g2_w2, g2_b2)
            m["xqt"] = x1[b].T.astype(np.float16, order="C")
        in_maps.append(m)
    return in_maps


def kernel(x1, x2, Wkv1, Wkv2, g1_w1, g1_b1, g1_w2, g1_b2,
           g2_w1, g2_b1, g2_w2, g2_b2, _runner=None):
    """Full-input entry point.  Returns (o1, o2), each [4, 4096, 1024] f32."""
    from concourse.bass_utils import run_bass_kernel_spmd

    args = [np.asarray(a, dtype=np.float32) for a in
            (x1, x2, Wkv1, Wkv2, g1_w1, g1_b1, g1_w2, g1_b2,
             g2_w1, g2_b1, g2_w2, g2_b2)]
    with_bias = bool(np.any(args[7]) or np.any(args[11]))  # g1_b2, g2_b2
    nc = _get_program(GATE_MODE, with_bias)
    in_maps = make_in_maps(*args)
    if not with_bias:
        for m in in_maps:
            m.pop("b2r", None)
    if _runner is None:
        res = run_bass_kernel_spmd(nc, in_maps, core_ids=list(range(8)))
        results = res.results
    else:
        results = _runner(nc, in_maps)

    B = x1.shape[0]
    o1 = np.empty((B, N, C), dtype=np.float32)
    o2 = np.empty((B, N, C), dtype=np.float32)
    for core in range(8):
        s, b = core // 4, core % 4
        arr = np.asarray(results[core]["ot"], dtype=np.float32)
        # tiled pieces [j, h, p, a, nn] (see phase B dma) -> [N, C]
        out = (arr.reshape(8, 2, 128, 2, 1024)
               .transpose(1, 3, 4, 0, 2).reshape(N, C))
        if s == 0:
            o2[b] = out   # core projected x1 -> ctx1 -> o2 = q2 @ ctx1
        else:
            o1[b] = out
    return (o1, o2)



# revision 60
# speedup vs baseline: 1.0068x; 1.0068x over previous
"""Trainium2 Bass kernel for nn_CrossAttention (dense_transformer).

Reference computation (per batch b, per stream s in {1,2}):
    q_s   = heads(x_s)                      # [H, N, D] slices of x_s
    kv_s  = x_s @ Wkv_s -> k_s, v_s         # [N, C] each
    gate_s= sigmoid(relu(x_s @ w1 + b1) @ w2 + b2)
    ctx_s = softmax_d( scale * k_s^T @ (v_s * gate_s) )   # [H, D, D]
    o_1   = q_1 @ ctx_2 ; o_2 = q_2 @ ctx_1  (cross)

Sharding: 8 cores = (stream s, batch b) pairs.  Core (s, b) projects
x_s[b] (kv + gate + ctx_s[b]) and then computes the OTHER stream's
output o_{1-s}[b] = q_{1-s}[b] @ softmax(ctx_s[b]).  No cross-core
communication; host concatenates outputs.

v2: host pre-transposes/pre-casts x (fp16), so the device does no
transposes and no DRAM spills.  All GEMMs fp16 (1 cycle/row); the gate
MLP can optionally run fp8e4 DoubleRow (0.5 cycles/row).  ctx is
accumulated in PSUM across all 32 n-chunks (two 8-head groups stacked
on partition halves -> one PSUM bank).
"""

import numpy as np
from contextlib import ExitStack

N = 4096
C = 1024
H = 16
D = 64
SCALE = D ** (-0.5)
NCH = N // 128       # 32 n-chunks of 128 rows

GATE_MODE = "fp8"    # 'fp16' | 'fp8'
S_X = 16.0           # fp8 activation scale for x
S_W = 256.0          # fp8 weight scale
S_H = 32.0           # fp8 scale for hidden h

_CACHE = {}


def _build_program(gate_mode, with_bias):
    import concourse.bass as bass
    import concourse.bacc as bacc
    import concourse.tile as tile
    import concourse.mybir as mybir

    F32 = mybir.dt.float32
    FP16 = mybir.dt.float16
    FP8 = mybir.dt.float8e4
    AF = mybir.ActivationFunctionType
    DR = mybir.MatmulPerfMode.DoubleRow
    fp8 = gate_mode == "fp8"
    HDT = FP8 if fp8 else FP16

    nc = bacc.Bacc("TRN2", target_bir_lowering=False, debug=False, num_devices=8)

    # weights arrive host-rearranged to the SBUF layout [p, k, m] so the
    # DMA is a straight contiguous copy (8-16KB lines per partition)
    xt = nc.dram_tensor("xt", [C, N], FP16, kind="ExternalInput").ap()
    xqt = nc.dram_tensor("xqt", [C, N], FP16, kind="ExternalInput").ap()
    wkv = nc.dram_tensor("wkv", [128, 8 * 2 * C], FP16, kind="ExternalInput").ap()
    w1 = nc.dram_tensor("w1", [128, 8 * C], HDT, kind="ExternalInput").ap()
    w2 = nc.dram_tensor("w2", [128, 8 * C], HDT, kind="ExternalInput").ap()
    b1s = nc.dram_tensor("b1s", [128, 8], F32, kind="ExternalInput").ap()
    ident = nc.dram_tensor("ident", [128, 64], F32, kind="ExternalInput").ap()
    if fp8:
        xt8 = nc.dram_tensor("xt8", [C, N], FP8, kind="ExternalInput").ap()
    if with_bias:
        b2r = nc.dram_tensor("b2r", [1, C], FP16, kind="ExternalInput").ap()
    # output in TILED layout: piece (j, pi) = [128 c-dims of pair j,
    # 1024 n-cols] stored fully contiguous (256KB DRAM bursts instead of
    # 2KB rows strided 8KB -> better write bandwidth); host reassembles.
    ot = nc.dram_tensor("ot", [N, C], FP16, kind="ExternalOutput").ap()
    # tiny sink so the act-table prewarm exps aren't dead-code eliminated
    wsink = nc.dram_tensor("wsink", [1, 16], F32, kind="ExternalOutput").ap()

    # activation post-scales to undo the fp8 pre-scales
    g1_scale = (S_H / (S_X * S_W)) if fp8 else 1.0
    g2_scale = (1.0 / (S_H * S_W)) if fp8 else 1.0
    ones_val = (S_H * S_W) if fp8 else 1.0

    with tile.TileContext(nc) as tc, ExitStack() as ctx:
        # ---------- persistent constants ----------
        # DMA order matters: only w1/b1 (+ first x block) gate the first
        # matmul, so emit those first and defer w2/wkv into block 0's
        # compute window.
        # big DMAs are split into k-range parts so they spread across DMA
        # queues (a single dma_start lands on one queue)
        def dma_split(dst, src, parts):
            kk = 8 // parts
            for i in range(parts):
                nc.sync.dma_start(
                    dst[:, i * kk:(i + 1) * kk, :], src[:, i * kk:(i + 1) * kk, :]
                )

        cpool = ctx.enter_context(tc.tile_pool(name="consts", bufs=1))
        w1_sb = cpool.tile([128, 8, C], HDT, name="w1_sb")
        b1_sb = cpool.tile([128, 8], F32, name="b1_sb")
        w2_sb = cpool.tile([128, 8, C], HDT, name="w2_sb")
        wkv_sb = cpool.tile([128, 8, 2 * C], FP16, name="wkv_sb")
        ident_sb = cpool.tile([128, 64], F32, name="ident_sb")

        def emit_deferred_consts():
            dma_split(w2_sb, w2.rearrange("p (k m) -> p k m", k=8), 4)
            dma_split(wkv_sb, wkv.rearrange("p (k m) -> p k m", k=8), 8)
            nc.sync.dma_start(ident_sb, ident)
        if with_bias:
            ones_sb = cpool.tile([1, 128], F32, name="ones_sb")
            nc.vector.memset(ones_sb, ones_val)
            ones_r = cpool.tile([1, 128], FP16, name="ones_r")
            nc.vector.tensor_copy(ones_r, ones_sb)
            b2_r = cpool.tile([1, C], FP16, name="b2_r")
            nc.sync.dma_start(b2_r, b2r)

        # pre-warm the scalar Exp activation table while the engine is idle
        # so the softmax doesn't pay the ~1.3us ACT_TABLE_LOAD at its start
        warm = cpool.tile([1, 16], F32, name="warm")
        nc.vector.memset(warm, 0.0)
        warm2 = cpool.tile([1, 16], F32, name="warm2")
        nc.scalar.activation(warm2, warm, AF.Exp)

        # spair off-diagonal blocks are zero forever: zero them once here,
        # off the critical path, so softmax only writes the diagonals
        spool = ctx.enter_context(tc.tile_pool(name="spairs", bufs=1))
        spairs = [spool.tile([128, 128], FP16, name=f"spair{j}") for j in range(8)]
        for j in range(8):
            nc.vector.memset(spairs[j], 0.0)

        # ctx accumulator in PSUM: heads 0-7 on partitions 0-63, heads
        # 8-15 on 64-127; head h at cols (h%8)*64, layout [e, d].
        ctxps_pool = ctx.enter_context(
            tc.tile_pool(name="ctxps", bufs=1, space="PSUM")
        )
        ctx_ps = ctxps_pool.tile([128, 512], F32, name="ctx_ps")

        # phase-B xq tiles live alongside phase A so DMA prefetch overlaps
        bxq_pool = ctx.enter_context(tc.tile_pool(name="bxq", bufs=4))

        def emit_bxq_dma(blk):
            bx = bxq_pool.tile([128, 8, 1024], FP16, name="bx", tag="bx")
            src = xqt.rearrange("(j p) n -> p j n", p=128)[
                :, :, blk * 1024:(blk + 1) * 1024
            ]
            for i in range(4):
                nc.sync.dma_start(bx[:, 2 * i:2 * i + 2, :], src[:, 2 * i:2 * i + 2, :])
            return bx

        # =========================================================
        # Phase A: gates + kv projection + ctx accumulation, fused
        # =========================================================
        with ExitStack() as pa:
            xt_pool = pa.enter_context(tc.tile_pool(name="xt", bufs=2))
            if fp8:
                xt8_pool = pa.enter_context(tc.tile_pool(name="xt8", bufs=2))
            ht_pool = pa.enter_context(tc.tile_pool(name="ht", bufs=2))
            g_pool = pa.enter_context(tc.tile_pool(name="g", bufs=3))
            kf_pool = pa.enter_context(tc.tile_pool(name="kf", bufs=4))
            vg_pool = pa.enter_context(tc.tile_pool(name="vg", bufs=4))
            gps_pool = pa.enter_context(
                tc.tile_pool(name="gps", bufs=3, space="PSUM")
            )
            kvps_pool = pa.enter_context(
                tc.tile_pool(name="kvps", bufs=2, space="PSUM")
            )

            bx_tiles = {}
            pending = []  # (kf, vg, global_chunk) awaiting ctx matmuls

            def emit_ctx(kf_t, vg_t, gc):
                # start=True marks the whole 2KB PSUM bank (per partition)
                # as pending-zero, so issue it exactly once per partition
                # half; the other heads' first writes then init via the
                # pending-zero overwrite instead of accumulating garbage.
                # Head order puts the softmax's first column stage (ctx cols
                # 0:256 = heads 0-3 and 8-11) first so its reduce can start
                # before the remaining heads' final matmuls land.  h=0 and
                # h=8 stay first within their partition halves, preserving
                # the start-flag invariant.
                for h in (0, 1, 2, 3, 8, 9, 10, 11, 4, 5, 6, 7, 12, 13, 14, 15):
                    nc.tensor.matmul(
                        ctx_ps[
                            (h // 8) * 64:(h // 8) * 64 + 64,
                            (h % 8) * 64:(h % 8) * 64 + 64,
                        ],
                        vg_t[:, h * D:(h + 1) * D],
                        kf_t[:, h * D:(h + 1) * D],
                        start=(gc == 0 and h % 8 == 0),
                        stop=(gc == NCH - 1),
                        skip_group_check=True,
                    )

            def emit_inputs(blk):
                # input DMAs split by n-half so the first matmul of the
                # block is gated on half the bytes; at blk 0 interleave the
                # w1/b1 emissions so all gating loads land on the earliest-
                # starting DMA queues.  Quarter-sized parts: the DMA
                # pipeline has a fixed ~8us startup latency and bigger
                # pieces ramp its throughput faster than many small ones;
                # all dynamic queues share one DMA engine, so everything
                # stays on sync's queue in consumption order.
                xt8_t = None
                if fp8:
                    xt8_t = xt8_pool.tile([128, 8, C], FP8, name="xt8_in", tag="xt8")
                    src8 = xt8.rearrange("(k p) n -> p k n", p=128)
                    w1m = w1.rearrange("p (k m) -> p k m", k=8)
                    # piece order matches gate1's all-m-over-half0 sweep:
                    # that sweep consumes xt8-h0 AND all four w1 quarters
                    # in its first 6.8us, while xt8-h1 isn't needed until
                    # the second sweep -- so h1 pieces go last at blk 0
                    def xt8_piece(kq, hf):
                        nc.sync.dma_start(
                            xt8_t[:, kq * 4:(kq + 1) * 4,
                                  hf * 512:(hf + 1) * 512],
                            src8[:, kq * 4:(kq + 1) * 4,
                                 blk * 1024 + hf * 512:
                                 blk * 1024 + (hf + 1) * 512],
                        )

                    def w1_piece(i):
                        nc.sync.dma_start(
                            w1_sb[:, :, i * 256:(i + 1) * 256],
                            w1m[:, :, i * 256:(i + 1) * 256],
                        )

                    if blk == 0:
                        w1_piece(0)
                        xt8_piece(0, 0)
                        xt8_piece(1, 0)
                        nc.sync.dma_start(b1_sb, b1s)
                        w1_piece(1)
                        w1_piece(2)
                        w1_piece(3)
                        xt8_piece(0, 1)
                        xt8_piece(1, 1)
                    else:
                        for hf in range(2):
                            for kq in range(2):
                                xt8_piece(kq, hf)
                elif blk == 0:
                    dma_split(w1_sb, w1.rearrange("p (k m) -> p k m", k=8), 8)
                    nc.sync.dma_start(b1_sb, b1s)
                xt_t = xt_pool.tile([128, 8, C], FP16, name="xt_in", tag="xt")
                srcx = xt.rearrange("(k p) n -> p k n", p=128)
                for hf in range(2):
                    for kq in range(2):
                        nc.sync.dma_start(
                            xt_t[:, kq * 4:(kq + 1) * 4,
                                 hf * 512:(hf + 1) * 512],
                            srcx[:, kq * 4:(kq + 1) * 4,
                                 blk * 1024 + hf * 512:
                                 blk * 1024 + (hf + 1) * 512],
                        )
                return xt8_t, xt_t

            in_tiles = {0: emit_inputs(0)}
            for blk in range(4):
                xt8_in, xt_in = in_tiles.pop(blk)

                # ---- gate1: hT[m-tile, n] = relu(x@w1+b1).T ----
                # half OUTER ACROSS m: the whole m-sweep for n-half 0 runs
                # on the first 512KB of xt8 (6.8us of compute) while the
                # h1 pieces stream in behind it -> no mid-gate1 DMA stall
                ht = ht_pool.tile([128, 8, C], HDT, name="ht", tag="ht")
                for half in range(2):
                    for m in range(8):
                        ps1 = gps_pool.tile([128, 512], F32, name="g1ps",
                                            tag="gps")
                        if fp8:
                            for kp in range(4):
                                nc.tensor.matmul(
                                    ps1,
                                    w1_sb[:, 2 * kp:2 * kp + 2,
                                          m * 128:(m + 1) * 128],
                                    xt8_in[:, 2 * kp:2 * kp + 2,
                                           half * 512:(half + 1) * 512],
                                    start=(kp == 0),
                                    stop=(kp == 3),
                                    perf_mode=DR,
                                )
                        else:
                            for k in range(8):
                                nc.tensor.matmul(
                                    ps1,
                                    w1_sb[:, k, m * 128:(m + 1) * 128],
                                    xt_in[:, k, half * 512:(half + 1) * 512],
                                    start=(k == 0),
                                    stop=(k == 7),
                                )
                        nc.scalar.activation(
                            ht[:, m, half * 512:(half + 1) * 512],
                            ps1,
                            AF.Relu,
                            bias=b1_sb[:, m:m + 1],
                            scale=g1_scale,
                        )

                if blk == 0:
                    # w2/wkv arrive during block 0's gate1; xq prefetches after
                    emit_deferred_consts()
                # prefetch phase-B xq tiles while DMA is quiet
                if blk >= 1:
                    bx_tiles[blk - 1] = emit_bxq_dma(blk - 1)
                    if blk == 3:
                        bx_tiles[3] = emit_bxq_dma(3)

                # ---- per chunk: gate2 -> kv -> (delayed) ctx ----
                for c in range(8):
                    gc = blk * 8 + c
                    if c == 3 and blk < 3:
                        # prefetch the next block's inputs from mid-block so
                        # its gate1 never waits on the just-issued DMAs
                        in_tiles[blk + 1] = emit_inputs(blk + 1)
                    gt = g_pool.tile([128, C], FP16, name="gt", tag="gt")
                    for t in range(2):
                        ps2 = gps_pool.tile([128, 512], F32, name="g2ps", tag="gps")
                        if fp8:
                            for kp in range(4):
                                nc.tensor.matmul(
                                    ps2,
                                    ht[:, 2 * kp:2 * kp + 2, c * 128:(c + 1) * 128],
                                    w2_sb[:, 2 * kp:2 * kp + 2,
                                          t * 512:(t + 1) * 512],
                                    start=(kp == 0),
                                    stop=(kp == 3 and not with_bias),
                                    perf_mode=DR,
                                )
                        else:
                            for k in range(8):
                                nc.tensor.matmul(
                                    ps2,
                                    ht[:, k, c * 128:(c + 1) * 128],
                                    w2_sb[:, k, t * 512:(t + 1) * 512],
                                    start=(k == 0),
                                    stop=(k == 7 and not with_bias),
                                )
                        if with_bias:
                            nc.tensor.matmul(
                                ps2,
                                ones_r,
                                b2_r[:, t * 512:(t + 1) * 512],
                                start=False,
                                stop=True,
                            )
                        nc.scalar.activation(
                            gt[:, t * 512:(t + 1) * 512], ps2, AF.Sigmoid,
                            scale=g2_scale,
                        )
                    if gc == NCH - 1:
                        # last sigmoid just ran: the act-table switch to the
                        # exp set (~1.3us) would otherwise land on the softmax
                        # critical path.  Force it now, hidden under the last
                        # chunk's kv matmuls (relu/copy/exp share a table set;
                        # only sigmoid conflicts).  Reading gt (the last
                        # sigmoid's output) pins this after that sigmoid in
                        # the schedule; the DMA sink defeats dead-code elim.
                        nc.scalar.activation(warm2, gt[0:1, 1008:1024], AF.Exp)
                        nc.sync.dma_start(wsink, warm2)

                    # kv projection for this chunk; k and v psum halves
                    ps_k = kvps_pool.tile([128, C], F32, name="ps_k", tag="kvps")
                    ps_v = kvps_pool.tile([128, C], F32, name="ps_v", tag="kvps")
                    for k in range(8):
                        lhs = xt_in[:, k, c * 128:(c + 1) * 128]
                        for t in range(2):
                            nc.tensor.matmul(
                                ps_k[:, t * 512:(t + 1) * 512],
                                lhs,
                                wkv_sb[:, k, t * 512:(t + 1) * 512],
                                start=(k == 0),
                                stop=(k == 7),
                            )
                        for t in range(2):
                            nc.tensor.matmul(
                                ps_v[:, t * 512:(t + 1) * 512],
                                lhs,
                                wkv_sb[:, k, C + t * 512:C + (t + 1) * 512],
                                start=(k == 0),
                                stop=(k == 7),
                            )
                    kf = kf_pool.tile([128, C], FP16, name="kf", tag="kf")
                    nc.scalar.copy(kf, ps_k)
                    vg = vg_pool.tile([128, C], FP16, name="vg", tag="vg")
                    if gc == NCH - 1:
                        # split the final vg mul so the last ctx pair (heads
                        # 0-7 need only cols 0:512) starts ~0.6us earlier
                        nc.vector.tensor_mul(vg[:, 0:512], ps_v[:, 0:512],
                                             gt[:, 0:512])
                        nc.vector.tensor_mul(vg[:, 512:1024], ps_v[:, 512:1024],
                                             gt[:, 512:1024])
                    else:
                        nc.vector.tensor_mul(vg, ps_v, gt)

                    # ctx lags a chunk and is emitted in pairs: the kf/vg
                    # conversions overlap the kv matmuls, and batching two
                    # chunks of tiny ctx matmuls halves the PE tile-mode
                    # transitions (~200ns each); lag kept minimal so the
                    # post-loop drain is at most two chunks
                    pending.append((kf, vg, gc))
                    if len(pending) >= 3:
                        emit_ctx(*pending.pop(0))
                        emit_ctx(*pending.pop(0))

            while pending:
                emit_ctx(*pending.pop(0))

        # =========================================================
        # Softmax over d (free dim of ctxT) + block-diag S pairs
        # =========================================================
        with ExitStack() as sm:
            smp = sm.enter_context(tc.tile_pool(name="smpool", bufs=1))
            # 4 bufs (not 7): leaves 3 PSUM banks free so phase B's first
            # matmuls aren't blocked on the spair copies draining these
            smps = sm.enter_context(tc.tile_pool(name="smps", bufs=4, space="PSUM"))
            maxs = smp.tile([128, 8], F32, name="maxs")
            cmx = smp.tile([128, 512], F32, name="cmx")
            et = smp.tile([128, 512], F32, name="et")
            sums = smp.tile([128, 8], F32, name="sums")
            recs = smp.tile([128, 8], F32, name="recs")
            st = smp.tile([128, 512], F32, name="st")
            # softmax runs in two column halves so the first transposes
            # (pairs 0,1,4,5 live in cols 0:256) start ~2us earlier
            for c0, c1, pairs in ((0, 256, (0, 1, 4, 5)),
                                  (256, 512, (2, 3, 6, 7))):
                cs = slice(c0, c1)
                hs = slice(c0 // 64, c1 // 64)
                b = (c1 - c0) // 64
                nc.vector.tensor_reduce(
                    maxs[:, hs],
                    ctx_ps[:, cs].rearrange("p (b d) -> p b d", b=b),
                    axis=mybir.AxisListType.X,
                    op=mybir.AluOpType.max,
                )
                nc.vector.tensor_sub(
                    cmx[:, cs].rearrange("p (h d) -> p h d", h=b),
                    ctx_ps[:, cs].rearrange("p (h d) -> p h d", h=b),
                    maxs[:, hs].unsqueeze(-1).broadcast_to([128, b, 64]),
                )
                nc.scalar.activation(et[:, cs], cmx[:, cs], AF.Exp,
                                     scale=float(SCALE))
                nc.vector.tensor_reduce(
                    sums[:, hs],
                    et[:, cs].rearrange("p (b d) -> p b d", b=b),
                    axis=mybir.AxisListType.X,
                    op=mybir.AluOpType.add,
                )
                nc.vector.reciprocal(recs[:, hs], sums[:, hs])
                nc.vector.tensor_mul(
                    st[:, cs].rearrange("p (h d) -> p h d", h=b),
                    et[:, cs].rearrange("p (h d) -> p h d", h=b),
                    recs[:, hs].unsqueeze(-1).broadcast_to([128, b, 64]),
                )
            # st rows e (64 per half), cols d per head.  Transposing the
            # side-by-side pair [ctxT_2j | ctxT_2j+1] ([64, 128]) gives
            # [S_2j stacked above S_2j+1] ([128, 64]); scatter block-diag.
            # Both copies of a pair stay on ONE engine (cross-engine writes
            # into the same tile raced); pair order follows softmax halves.
            for i, j in enumerate((0, 1, 4, 5, 2, 3, 6, 7)):
                half = j // 4  # heads 0-7 lower partitions, 8-15 upper
                base = half * 64
                colj = (2 * j) % 8
                tp = smps.tile([128, 64], F32, name="smtp", tag="smtp")
                nc.tensor.transpose(
                    tp,
                    st[base:base + 64, colj * 64:(colj + 2) * 64],
                    ident_sb[base:base + 64, :],
                )
                # pairs 0,1 (phase B's first two) copy on SCALAR, which is
                # free right after its exps; vector is still draining the
                # stage-2 softmax ops, so putting the early pairs there
                # delays phase B's first matmuls
                if j in (0, 1):
                    nc.scalar.copy(spairs[j][0:64, 0:64], tp[0:64, :])
                    nc.scalar.copy(spairs[j][64:128, 64:128], tp[64:128, :])
                else:
                    nc.vector.tensor_copy(spairs[j][0:64, 0:64], tp[0:64, :])
                    nc.vector.tensor_copy(spairs[j][64:128, 64:128], tp[64:128, :])

        # =========================================================
        # Phase B: ot[j*128:(j+1)*128, n] = blockdiag(S_j)^T @ q_pair
        # S stationary per j: 8 matmuls of free=512 each (vs 256 tiny
        # ones), paced by PSUM->SBUF copies spread over three engines.
        # =========================================================
        with ExitStack() as pb:
            oj_pool = pb.enter_context(tc.tile_pool(name="bo", bufs=3))
            bops_pool = pb.enter_context(
                tc.tile_pool(name="bops", bufs=7, space="PSUM")
            )
            ei = 0
            # GPSIMD cannot read PSUM on TRN2: strictly alternate the two
            # engines that can, so each (vector, scalar) pair of copies
            # completes in one copy-time and feeds the output DMA steadily
            for j in (0, 1, 4, 5, 2, 3, 6, 7):
                oj = oj_pool.tile([128, N], FP16, name="oj", tag="oj")
                for nb in range(8):
                    blk, half = nb // 2, nb % 2
                    ops = bops_pool.tile([128, 512], F32, name="ops", tag="ops")
                    nc.tensor.matmul(
                        ops,
                        spairs[j],
                        bx_tiles[blk][:, j, half * 512:(half + 1) * 512],
                        start=True,
                        stop=True,
                        skip_group_check=True,
                    )
                    # 34:30 vector:scalar — scalar's copies slow from 687 to
                    # ~880ns over phase B, so shift two late-j copies to the
                    # vector engine to keep the two queues draining together.
                    # j0 starts on SCALAR: vector is still draining softmax
                    # stage-2 ops + spair copies, scalar is free after its
                    # exps, so the first output piece forms ~1us earlier.
                    if j == 0:
                        use_vec = ei % 2 == 1
                    else:
                        use_vec = ei % 2 == 0 or (j in (6, 7) and nb == 7)
                    if use_vec:
                        nc.vector.tensor_copy(oj[:, nb * 512:(nb + 1) * 512], ops)
                    else:
                        nc.scalar.copy(oj[:, nb * 512:(nb + 1) * 512], ops)
                    ei += 1
                    if nb % 4 == 3:
                        # 512KB output pieces: sync's sequencer issues one
                        # DMA per ~1us, so 16 pieces keep issue capacity
                        # (~512GB/s) above the ~300GB/s write bandwidth;
                        # each piece lands contiguous in DRAM (tiled layout)
                        pi = j * 2 + nb // 4
                        nc.sync.dma_start(
                            ot[pi * 256:(pi + 1) * 256, :].rearrange(
                                "(p a) n -> p (a n)", a=2),
                            oj[:, (nb - 3) * 512:(nb + 1) * 512],
                        )

    nc.compile()
    return nc


def _get_program(gate_mode=None, with_bias=False):
    if gate_mode is None:
        gate_mode = GATE_MODE
    key = (gate_mode, bool(with_bias))
    if key not in _CACHE:
        _CACHE[key] = _build_program(gate_mode, with_bias)
    return _CACHE[key]


def make_in_maps(x1, x2, Wkv1, Wkv2, g1_w1, g1_b1, g1_w2, g1_b2,
                 g2_w1, g2_b1, g2_w2, g2_b2, gate_mode=None):
    """Core (s, b): cores 0-3 = (s=0, b), cores 4-7 = (s=1, b)."""
    import ml_dtypes
    if gate_mode is None:
        gate_mode = GATE_MODE
    fp8 = gate_mode == "fp8"
    F8 = ml_dtypes.float8_e4m3
    ident = np.vstack([np.eye(64, dtype=np.float32)] * 2)

    def dev_w(w):
        # [k*128+p, m] -> [p, k*M+m] (SBUF layout, contiguous DMA lines)
        M = w.shape[1]
        return np.ascontiguousarray(
            w.reshape(8, 128, M).transpose(1, 0, 2).reshape(128, 8 * M)
        )

    def prep_stream(x, wkv, w1, b1, w2, b2):
        m = {
            "xt": x.T.astype(np.float16, order="C"),
            "wkv": dev_w(wkv.astype(np.float16)),
            "ident": ident,
        }
        if fp8:
            m["xt8"] = (x.T * S_X).astype(F8, order="C")
            m["w1"] = dev_w((w1 * S_W).astype(F8))
            m["w2"] = dev_w((w2 * S_W).astype(F8))
            m["b1s"] = np.ascontiguousarray((S_H * b1).reshape(8, 128).T)
        else:
            m["w1"] = dev_w(w1.astype(np.float16))
            m["w2"] = dev_w(w2.astype(np.float16))
            m["b1s"] = np.ascontiguousarray(b1.reshape(8, 128).T)
        m["b2r"] = b2.reshape(1, C).astype(np.float16)
        return m

    in_maps = []
    for core in range(8):
        s, b = core // 4, core % 4
        if s == 0:
            m = prep_stream(x1[b], Wkv1, g1_w1, g1_b1, g1_w2, g1_b2)
            m["xqt"] = x2[b].T.astype(np.float16, order="C")
        else:
            m = prep_stream(x2[b], Wkv2, g2_w1, g2_b1, g2_w2, g2_b2)
            m["xqt"] = x1[b].T.astype(np.float16, order="C")
        in_maps.append(m)
    return in_maps


def kernel(x1, x2, Wkv1, Wkv2, g1_w1, g1_b1, g1_w2, g1_b2,
           g2_w1, g2_b1, g2_w2, g2_b2, _runner=None):
    """Full-input entry point.  Returns (o1, o2), each [4, 4096, 1024] f32."""
    from concourse.bass_utils import run_bass_kernel_spmd

    args = [np.asarray(a, dtype=np.float32) for a in
            (x1, x2, Wkv1, Wkv2, g1_w1, g1_b1, g1_w2, g1_b2,
             g2_w1, g2_b1, g2_w2, g2_b2)]
    with_bias = bool(np.any(args[7]) or np.any(args[11]))  # g1_b2, g2_b2
    nc = _get_program(GATE_MODE, with_bias)
    in_maps = make_in_maps(*args)
    if not with_bias:
        for m in in_maps:
            m.pop("b2r", None)
    if _runner is None:
        res = run_bass_kernel_spmd(nc, in_maps, core_ids=list(range(8)))
        results = res.results
    else:
        results = _runner(nc, in_maps)

    B = x1.shape[0]
    o1 = np.empty((B, N, C), dtype=np.float32)
    o2 = np.empty((B, N, C), dtype=np.float32)
    for core in range(8):
        s, b = core // 4, core % 4
        arr = np.asarray(results[core]["ot"], dtype=np.float32)
        # tiled pieces [j, h, p, a, nn] (see phase B dma) -> [N, C]
        out = (arr.reshape(8, 2, 128, 2, 1024)
               .transpose(1, 3, 4, 0, 2).reshape(N, C))
        if s == 0:
            o2[b] = out   # core projected x1 -> ctx1 -> o2 = q2 @ ctx1
        else:
            o1[b] = out
    return (o1, o2)

